# revision 12
# baseline (speedup 1.0000x reference)
"""Batched multi-head graph attention (GAT) kernel for 8 Trainium2 NeuronCores.

Math (per batch b, head h):
    hp      = h[b] @ w[h]                          # [N, F]
    t       = tanh(hp)
    s       = t @ a_src[h];  d = t @ a_dst[h]      # [N]
    score   = leaky_relu(s_i + d_j, 0.2)
    e       = where(adj>0, exp(score), 0)
    out     = (e / e.sum(-1, keepdim)) @ hp + bias

Identities used on-device (slope 0.2 < 1):
    exp(leaky(z)) = max(exp(z), exp(0.2 z))
                  = e^{0.2 s_i} * max(e^{0.8 s_i} e^{d_j}, e^{0.2 d_j})
    The e^{0.2 s_i} factor is row-constant -> cancels in softmax.
    With q=e^{0.8s}, v=e^d, v2=e^{0.2d}, the masked weight per 128x1024 tile:
        M[j,i] = adjT[j,i] * max(q_i v_j, v2_j)
    Two unit flavors, assigned per key-block (ACT_JBS):
      dve: ea = (q*v) max v2 on DVE tensor_scalar; M = ea*adjT on DVE TT.
      act: X = q*adjT on DVE TT (premask); M_relu = relu(v*X - c*v2) on the
           ACT engine (c = 1.875, the mask's surviving value, so that
           relu(v*(c q) - c v2) = c relu(qv - v2) exactly; zero rows stay 0);
           the missing v2 part is restored by a PE matmul with v2-scaled
           [hp|1] weights against the raw adjT.
    A single PE matmul per tile against lhsT=[hp | 1] accumulates numerator
    and denominator together into PSUM [65, 512].

adj mask trick: adj values are exactly 0.0/1.0 fp32; the high uint16 halves
read as fp16 equal 0.0/1.875 -- a constant scale on every surviving term,
which cancels in the normalization.  The host passes the high halves of the
TRANSPOSED adjacency (adjT[key, row]), so the device DMA is fully contiguous.

s is computed in row form via PE (hpT = w^T @ h^T, tanh, then a_src^T @ tanhT)
instead of block-form products + transposes; d stays in block (per-partition)
form via an elementwise product + reduce.

Sharding: 8 cores = 4 batches x 2 query-row halves; each core handles all 4
heads for its 1024 query rows against all 2048 keys.  Keys are rotated on
host so each core's queries are local rows [0, 1024).
"""

import os
from contextlib import ExitStack

import numpy as np

import concourse.bass as bass
import concourse.mybir as mybir
import concourse.tile as tile
from concourse import bacc
from concourse.bass_utils import run_bass_kernel_spmd
from concourse.masks import make_identity

F32 = mybir.dt.float32
F16 = mybir.dt.float16
U16 = mybir.dt.uint16
ALU = mybir.AluOpType
ACTF = mybir.ActivationFunctionType
AX = mybir.AxisListType

B, N, H, F = 4, 2048, 4, 64
NCORES = 8
ROWS = N // 2          # query rows per core
KEYS = N               # keys per core (full)
NEG_SLOPE = 0.2
CMASK = 1.875          # fp16 value of the surviving-mask bitcast


def default_act_jbs(kb, frac=0.5):
    """Key-block indices routed through the ACT-engine relu path.

    Avoids the first and last block: the first so the loop leads with the
    cheap dve path while queues fill, the last so the accumulation closes
    without the extra v2 matmul on the critical tail.
    """
    n = int(round(kb * frac))
    if n <= 0:
        return set()
    inner = list(range(1, kb - 1))
    step = len(inner) / n
    return {inner[min(len(inner) - 1, int(i * step + step / 2))]
            for i in range(n)}


def build_program(rows=ROWS, keys=KEYS, heads=H, f=F, act_frac=0.5):
    nc = bacc.Bacc("TRN2", target_bir_lowering=False, debug=False)

    kb = keys // 128          # key blocks
    nhalf = max(1, rows // 512)   # output column chunks (psum tiles per head)
    hw = rows // nhalf        # columns per acc tile (512)
    fe = f + 1                # hp with ones column appended
    nchunk = max(1, rows // 512)  # chunks for hpT/srow row-form work

    act_jbs = default_act_jbs(kb, act_frac)

    hb_d = nc.dram_tensor("hb", [keys, f], F32, kind="ExternalInput")
    adjt_d = nc.dram_tensor("adjt", [keys, rows], U16, kind="ExternalInput")
    w_d = nc.dram_tensor("wmat", [heads, f, f], F32, kind="ExternalInput")
    ap_d = nc.dram_tensor("apairt", [heads, 2, f], F32, kind="ExternalInput")
    out_d = nc.dram_tensor("out", [heads, rows, f], F32,
                           kind="ExternalOutput")

    pairs = [tuple(range(p, min(p + 2, heads)))
             for p in range(0, heads, 2)]

    with tile.TileContext(nc) as tc:
        with (
            tc.tile_pool(name="const", bufs=1) as const,
            tc.tile_pool(name="persist", bufs=1) as persist,
            tc.tile_pool(name="stmp", bufs=4) as stmp,
        ):
            id16 = const.tile([128, 128], F16, tag="id16")
            make_identity(nc, id16)
            id32 = const.tile([128, 128], F32, tag="id32")
            make_identity(nc, id32)

            # ---- global loads -------------------------------------------
            h32 = persist.tile([128, kb, f], F32, tag="h32")
            nc.sync.dma_start(
                out=h32, in_=hb_d.ap().rearrange("(t p) f -> p t f", p=128))
            h16 = persist.tile([128, kb, f], F16, tag="h16")
            nc.vector.tensor_copy(h16, h32)

            w32 = persist.tile([f, heads, f], F32, tag="w32")
            nc.sync.dma_start(out=w32, in_=w_d.ap().rearrange("h f o -> f h o"))
            w16 = persist.tile([f, heads, f], F16, tag="w16")
            nc.vector.tensor_copy(w16, w32)

            # a_src transposed for row-form s matmuls: [f, heads]
            asT16 = persist.tile([f, heads], F16, tag="asT16")
            asT32 = persist.tile([f, heads], F32, tag="asT32")
            nc.sync.dma_start(
                out=asT32, in_=ap_d.ap()[:, 0, :].rearrange("h f -> f h"))
            nc.vector.tensor_copy(asT16, asT32)

            # a_dst broadcast for block-form d products: [128, heads, f]
            ad32 = persist.tile([1, heads, f], F32, tag="ad32")
            nc.sync.dma_start(out=ad32, in_=ap_d.ap()[:, 1, :].unsqueeze(0))
            adb32 = persist.tile([128, heads, f], F32, tag="adb32")
            nc.gpsimd.partition_broadcast(adb32, ad32)
            adb16 = persist.tile([128, heads, f], F16, tag="adb16")
            nc.vector.tensor_copy(adb16, adb32)

            # ---- mask tiles: contiguous DMA of host-transposed adjT ------
            with tc.tile_pool(name="adjp", bufs=kb) as adjp:
                adjts = []
                for jb in range(kb):
                    adjt = adjp.tile([128, rows], U16, tag="adjt",
                                     name=f"adjt{jb}")
                    nc.sync.dma_start(
                        out=adjt,
                        in_=adjt_d.ap()[jb * 128:(jb + 1) * 128, :])
                    adjts.append(adjt)

                # ---- hT (transposed h, fp16) ----------------------------
                hT16 = persist.tile([64, keys], F16, tag="hT16")
                g_ht = min(4, kb)
                with tc.tile_pool(name="psum_ht", bufs=2,
                                  space="PSUM") as pht:
                    for g in range(kb // g_ht):
                        pt = pht.tile([64, g_ht * 128], F16, tag="pht")
                        for t in range(g_ht):
                            blk = g * g_ht + t
                            nc.tensor.transpose(
                                pt[:, t * 128:(t + 1) * 128],
                                h16[:, blk, :], id16)
                        nc.vector.tensor_copy(
                            hT16[:, g * g_ht * 128:(g + 1) * g_ht * 128], pt)

                # ---- per-head state --------------------------------------
                hpt = [None] * heads    # [128, kb, fe] fp16: [hp | 1]
                hpt2 = [None] * heads   # [128, kb, fe] fp16: v2-scaled
                vv = [None] * heads     # [128, kb] f32: e^d
                v2v = [None] * heads    # [128, kb] f32: e^{0.2 d}
                nv2c = [None] * heads   # [128, kb] f32: -c * e^{0.2 d}
                qbcp = [persist.tile([128, len(pr), rows], F16,
                                     tag=f"qbcp{pi}", name=f"qbcp{pi}")
                        for pi, pr in enumerate(pairs)]

                setup_pools = ExitStack()
                php = setup_pools.enter_context(
                    tc.tile_pool(name="psum_hp", bufs=2, space="PSUM"))
                rowp = setup_pools.enter_context(
                    tc.tile_pool(name="psum_row", bufs=1, space="PSUM"))

                def setup_head_steps(h, sd_eng, hpt2_eng):
                    """Emit-steps (closures) for head h's setup; callable one
                    at a time so pair-1 head setup can be spread across the
                    pair-0 main loop without clumping any engine's queue."""
                    pi, k = h // 2, h % 2
                    hpt_h = persist.tile([128, kb, fe], F16, tag=f"hpt{h}",
                                         name=f"hpt{h}")
                    tanh_h = stmp.tile([128, kb, f], F16, tag="tanh",
                                       name=f"tanh{h}")
                    v_h = persist.tile([128, kb], F32, tag=f"v{h}",
                                       name=f"v{h}")
                    v2_h = persist.tile([128, kb], F32, tag=f"v2{h}",
                                        name=f"v2{h}")
                    nv2_h = persist.tile([128, kb], F32, tag=f"nv2{h}",
                                         name=f"nv2{h}")
                    sums = stmp.tile([128, kb, 1], F32, tag="sums",
                                     name=f"sums{h}")
                    tanhT = stmp.tile([64, rows], F16, tag="tanhT",
                                      name=f"tanhT{h}")
                    qrow = stmp.tile([1, rows], F16, tag="qrow",
                                     name=f"qrow{h}")
                    hpt[h] = hpt_h
                    vv[h], v2v[h], nv2c[h] = v_h, v2_h, nv2_h

                    steps = []
                    g_hp = min(8, kb)

                    def hp_group(g):
                        pp = php.tile([128, g_hp * f], F32, tag="php",
                                      name=f"php{h}_{g}")
                        for t in range(g_hp):
                            blk = g * g_hp + t
                            nc.tensor.matmul(
                                pp[:, t * f:(t + 1) * f],
                                lhsT=hT16[:, blk * 128:(blk + 1) * 128],
                                rhs=w16[:, h, :], start=True, stop=True)
                        # hp to fp16 on DVE, tanh on ACT (splits the load)
                        nc.vector.tensor_copy(
                            hpt_h[:, g * g_hp:(g + 1) * g_hp, 0:f],
                            pp.rearrange("p (t o) -> p t o", o=f))
                        nc.scalar.activation(
                            tanh_h[:, g * g_hp:(g + 1) * g_hp, :],
                            pp.rearrange("p (t o) -> p t o", o=f),
                            ACTF.Tanh)

                    for g in range(kb // g_hp):
                        steps.append(lambda g=g: hp_group(g))
                    steps.append(
                        lambda: nc.vector.memset(hpt_h[:, :, f:fe], 1.0))

                    # d = tanh . a_dst per key (block/partition form)
                    def d_prod():
                        prod = stmp.tile([128, kb, f], F16, tag="prod",
                                         name=f"prod{h}")
                        sd_eng.tensor_tensor(
                            out=prod, in0=tanh_h,
                            in1=adb16[:, h].unsqueeze(1).broadcast_to(
                                [128, kb, f]),
                            op=ALU.mult)
                        nc.vector.reduce_sum(sums, prod, axis=AX.X)

                    steps.append(d_prod)

                    def d_exps():
                        nc.scalar.activation(v_h, sums[:, :, 0], ACTF.Exp)
                        nc.scalar.activation(v2_h, sums[:, :, 0], ACTF.Exp,
                                             scale=NEG_SLOPE)
                        nc.vector.tensor_scalar_mul(nv2_h, v2_h, -CMASK)

                    steps.append(d_exps)

                    # s in row form: hpT = w^T @ h^T, tanh, a_src^T @ tanhT
                    def s_chunk(cch):
                        sl = slice(cch * hw, (cch + 1) * hw)
                        hpTp = rowp.tile([64, hw], F32, tag="hpT",
                                         name=f"hpT{h}_{cch}")
                        nc.tensor.matmul(
                            hpTp, lhsT=w16[:, h, :], rhs=hT16[:, sl],
                            start=True, stop=True)
                        nc.scalar.activation(tanhT[:, sl], hpTp, ACTF.Tanh)
                        srowp = rowp.tile([1, hw], F32, tag="srow",
                                          name=f"srow{h}_{cch}")
                        nc.tensor.matmul(
                            srowp, lhsT=asT16[:, h:h + 1], rhs=tanhT[:, sl],
                            start=True, stop=True)
                        # q = e^{0.8 s}
                        nc.scalar.activation(qrow[:, sl], srowp, ACTF.Exp,
                                             scale=1.0 - NEG_SLOPE)

                    for cch in range(nchunk):
                        steps.append(lambda c=cch: s_chunk(c))
                    steps.append(lambda: nc.gpsimd.partition_broadcast(
                        qbcp[pi][:, k, :], qrow))

                    # v2-scaled [hp | 1] for the act-path restore matmul
                    if act_jbs:
                        hpt2_h = persist.tile([128, kb, fe], F16,
                                              tag=f"hpt2{h}",
                                              name=f"hpt2{h}")
                        hpt2[h] = hpt2_h

                        def mk_hpt2():
                            hpt2_eng.tensor_tensor(
                                out=hpt2_h, in0=hpt_h,
                                in1=v2_h.unsqueeze(2).broadcast_to(
                                    [128, kb, fe]),
                                op=ALU.mult)

                        steps.append(mk_hpt2)
                    return steps

                for st in setup_head_steps(0, nc.vector, nc.vector):
                    st()
                for st in setup_head_steps(1, nc.vector, nc.vector):
                    st()
                late_steps = []

                # ---- main loop ------------------------------------------
                nacc = heads * nhalf
                acc_sb = persist.tile([fe, nacc, hw], F32, tag="acc_sb")

                with (
                    tc.tile_pool(name="mt", bufs=4) as mt,
                    tc.tile_pool(name="outp", bufs=4) as outp,
                ):
                    for pi, pair in enumerate(pairs):
                        np_ = len(pair)
                        acc_st = ExitStack()
                        accp = acc_st.enter_context(
                            tc.tile_pool(name=f"accp{pi}", bufs=1,
                                         space="PSUM"))
                        accs = {}
                        for h in pair:
                            for half in range(nhalf):
                                i = h * nhalf + half
                                accs[i] = accp.tile(
                                    [fe, hw], F32, tag=f"acc{i}",
                                    name=f"acc{i}")
                        for jb in range(kb):
                            if pi == 0 and jb == 1 and len(pairs) > 1:
                                late_steps = (
                                    setup_head_steps(2, nc.gpsimd,
                                                     nc.gpsimd)
                                    + setup_head_steps(3, nc.gpsimd,
                                                       nc.gpsimd))
                            if pi == 0 and jb >= 1:
                                # heads 2,3 setup rides under pair 0's loop,
                                # a step or two per iteration so no engine
                                # queue gets a long setup clump
                                per_jb = max(
                                    1, -(-len(late_steps) // max(1, kb - 4)))
                                for _ in range(per_jb):
                                    if late_steps:
                                        late_steps.pop(0)()
                            adj16 = adjts[jb].bitcast(F16)
                            is_act = jb in act_jbs
                            ta = mt.tile([128, np_, rows], F16, tag="ta")
                            tb = mt.tile([128, np_, rows], F16, tag="tb")
                            if is_act:
                                # X = q * adjT, then relu(v X - c v2) on ACT
                                nc.vector.tensor_tensor(
                                    out=ta, in0=qbcp[pi],
                                    in1=adj16.unsqueeze(1).broadcast_to(
                                        [128, np_, rows]),
                                    op=ALU.mult)
                                for k, h in enumerate(pair):
                                    nc.scalar.activation(
                                        tb[:, k, :], ta[:, k, :], ACTF.Relu,
                                        bias=nv2c[h][:, jb:jb + 1],
                                        scale=vv[h][:, jb:jb + 1])
                            else:
                                for k, h in enumerate(pair):
                                    nc.vector.tensor_scalar(
                                        out=ta[:, k, :], in0=qbcp[pi][:, k, :],
                                        scalar1=vv[h][:, jb:jb + 1],
                                        scalar2=v2v[h][:, jb:jb + 1],
                                        op0=ALU.mult, op1=ALU.max)
                                nc.vector.tensor_tensor(
                                    out=tb, in0=ta,
                                    in1=adj16.unsqueeze(1).broadcast_to(
                                        [128, np_, rows]),
                                    op=ALU.mult)
                            for k, h in enumerate(pair):
                                for half in range(nhalf):
                                    sl = slice(half * hw, (half + 1) * hw)
                                    nc.tensor.matmul(
                                        accs[h * nhalf + half],
                                        lhsT=hpt[h][:, jb, :],
                                        rhs=tb[:, k, sl],
                                        start=(jb == 0),
                                        stop=(jb == kb - 1 and not is_act))
                                    if is_act:
                                        nc.tensor.matmul(
                                            accs[h * nhalf + half],
                                            lhsT=hpt2[h][:, jb, :],
                                            rhs=adj16[:, sl],
                                            start=False,
                                            stop=(jb == kb - 1))

                        while late_steps:
                            late_steps.pop(0)()
                        # spill this pair's accumulators to SBUF on ACT
                        for h in pair:
                            for half in range(nhalf):
                                i = h * nhalf + half
                                nc.scalar.activation(
                                    acc_sb[:, i, :], accs[i], ACTF.Identity)
                        acc_st.close()
                        if pi == 0:
                            # setup psum pools (under accp0 on the stack) can
                            # only pop after accp0 does
                            setup_pools.close()
                        # normalize in transposed [i, o] form
                        nq = hw // 128
                        ptf_st = ExitStack()
                        ptf = ptf_st.enter_context(
                            tc.tile_pool(name=f"ptf{pi}", bufs=2,
                                         space="PSUM"))
                        for h in pair:
                            for half in range(nhalf):
                                i = h * nhalf + half
                                pt = ptf.tile([128, nq, fe], F32,
                                              tag=f"pt{pi}")
                                for q in range(nq):
                                    nc.tensor.transpose(
                                        pt[:, q, :],
                                        acc_sb[:, i, q * 128:(q + 1) * 128],
                                        id32[0:fe, 0:fe])
                                rcol = outp.tile([128, nq], F32, tag="rcol")
                                nc.vector.reciprocal(rcol, pt[:, :, f])
                                osb = outp.tile([128, nq, f], F32, tag="osb")
                                nc.vector.tensor_tensor(
                                    out=osb, in0=pt[:, :, 0:f],
                                    in1=rcol.unsqueeze(2).broadcast_to(
                                        [128, nq, f]),
                                    op=ALU.mult)
                                nc.sync.dma_start(
                                    out=out_d.ap()[
                                        h, half * hw:(half + 1) * hw, :]
                                    .rearrange("(q p) f -> p q f", p=128),
                                    in_=osb)
                        ptf_st.close()
    nc.compile()
    return nc


_PROGRAM_CACHE = {}


def _get_program():
    key = "full"
    if key not in _PROGRAM_CACHE:
        _PROGRAM_CACHE[key] = build_program()
    return _PROGRAM_CACHE[key]


def make_in_maps(h, adj, w, a_src, a_dst):
    """Shard + marshal the full inputs into 8 per-core input maps."""
    h = np.ascontiguousarray(np.asarray(h, dtype=np.float32))
    adj = np.ascontiguousarray(np.asarray(adj, dtype=np.float32))
    w = np.ascontiguousarray(np.asarray(w, dtype=np.float32))
    apairt = np.ascontiguousarray(
        np.concatenate([np.asarray(a_src)[:, None, :, 0],
                        np.asarray(a_dst)[:, None, :, 0]],
                       axis=1).astype(np.float32))  # [H, 2, F]
    in_maps = []
    for c in range(NCORES):
        b, r0 = c // 2, (c % 2) * ROWS
        hb = np.concatenate([h[b, r0:], h[b, :r0]], axis=0)  # rotate keys
        adj_rows = adj[b, r0:r0 + ROWS]
        adj_rot = np.concatenate([adj_rows[:, r0:], adj_rows[:, :r0]], axis=1)
        adjt_f = np.ascontiguousarray(adj_rot.T)  # [KEYS, ROWS] f32
        adjt = np.ascontiguousarray(
            adjt_f.view(np.uint16).reshape(KEYS, ROWS, 2)[:, :, 1])
        in_maps.append({
            "hb": np.ascontiguousarray(hb),
            "adjt": adjt,
            "wmat": w,
            "apairt": apairt,
        })
    return in_maps


def assemble_output(results, bias):
    """Gather per-core [H, ROWS, F] results into [B, H, N, F]."""
    out = np.empty((B, H, N, F), dtype=np.float32)
    for c in range(NCORES):
        b, r0 = c // 2, (c % 2) * ROWS
        out[b, :, r0:r0 + ROWS, :] = results[c]["out"]
    if bias is not None:
        out = out + np.asarray(bias, dtype=np.float32)[None, None, None, :]
    return out


def run(h, adj, w, a_src, a_dst, bias, trace=False, trace_kwargs=None):
    nc = _get_program()
    in_maps = make_in_maps(h, adj, w, a_src, a_dst)
    res = run_bass_kernel_spmd(nc, in_maps, core_ids=list(range(NCORES)),
                               trace=trace, **(trace_kwargs or {}))
    return assemble_output(res.results, bias), res


def kernel(h, adj, w, a_src, a_dst, bias):
    out, _ = run(h, adj, w, a_src, a_dst, bias,
                 trace=bool(int(os.environ.get("GAT_TRACE", "0"))))
    return out


# revision 17
# speedup vs baseline: 1.1763x; 1.1763x over previous
"""Batched multi-head graph attention (GAT) kernel for 8 Trainium2 NeuronCores.

Math (per batch b, head h):
    hp      = h[b] @ w[h]                          # [N, F]
    t       = tanh(hp)
    s       = t @ a_src[h];  d = t @ a_dst[h]      # [N]
    score   = leaky_relu(s_i + d_j, 0.2)
    e       = where(adj>0, exp(score), 0)
    out     = (e / e.sum(-1, keepdim)) @ hp + bias

Identities used on-device (slope 0.2 < 1):
    exp(leaky(z)) = max(exp(z), exp(0.2 z))
                  = e^{0.2 s_i} * max(e^{0.8 s_i} e^{d_j}, e^{0.2 d_j})
    The e^{0.2 s_i} factor is row-constant -> cancels in softmax.
    With q=e^{0.8s}, v=e^d, v2=e^{0.2d}, the masked weight per 128x1024 tile:
        M[j,i] = adjT[j,i] * max(q_i v_j, v2_j)
    Two unit flavors, assigned per key-block (ACT_JBS):
      dve: ea = (q*v) max v2 on DVE tensor_scalar; M = ea*adjT on DVE TT.
      act: X = q*adjT on DVE TT (premask); M_relu = relu(v*X - c*v2) on the
           ACT engine (c = 1.875, the mask's surviving value, so that
           relu(v*(c q) - c v2) = c relu(qv - v2) exactly; zero rows stay 0);
           the missing v2 part is restored by a PE matmul with v2-scaled
           [hp|1] weights against the raw adjT.
    A single PE matmul per tile against lhsT=[hp | 1] accumulates numerator
    and denominator together into PSUM [65, 512].

adj mask trick: adj values are exactly 0.0/1.0 fp32; the high uint16 halves
read as fp16 equal 0.0/1.875 -- a constant scale on every surviving term,
which cancels in the normalization.  The host passes the high halves of the
TRANSPOSED adjacency (adjT[key, row]), so the device DMA is fully contiguous.

s is computed in row form via PE (hpT = w^T @ h^T, tanh, then a_src^T @ tanhT)
instead of block-form products + transposes; d stays in block (per-partition)
form via an elementwise product + reduce.

Sharding: 8 cores = 4 batches x 2 query-row halves; each core handles all 4
heads for its 1024 query rows against all 2048 keys.  Keys are rotated on
host so each core's queries are local rows [0, 1024).
"""

import os
from contextlib import ExitStack

import numpy as np

import concourse.bass as bass
import concourse.mybir as mybir
import concourse.tile as tile
from concourse import bacc
from concourse.bass_utils import run_bass_kernel_spmd
from concourse.masks import make_identity

F32 = mybir.dt.float32
F16 = mybir.dt.float16
U16 = mybir.dt.uint16
ALU = mybir.AluOpType
ACTF = mybir.ActivationFunctionType
AX = mybir.AxisListType

B, N, H, F = 4, 2048, 4, 64
NCORES = 8
ROWS = N // 2          # query rows per core
KEYS = N               # keys per core (full)
NEG_SLOPE = 0.2
CMASK = 1.875          # fp16 value of the surviving-mask bitcast


def default_act_jbs(kb, frac=0.5):
    """Key-block indices routed through the ACT-engine relu path.

    Avoids the first and last block: the first so the loop leads with the
    cheap dve path while queues fill, the last so the accumulation closes
    without the extra v2 matmul on the critical tail.
    """
    n = int(round(kb * frac))
    if n <= 0:
        return set()
    inner = list(range(1, kb - 1))
    step = len(inner) / n
    return {inner[min(len(inner) - 1, int(i * step + step / 2))]
            for i in range(n)}


def build_program(rows=ROWS, keys=KEYS, heads=H, f=F, act_frac=0.5):
    nc = bacc.Bacc("TRN2", target_bir_lowering=False, debug=False)

    kb = keys // 128          # key blocks
    nhalf = max(1, rows // 512)   # output column chunks (psum tiles per head)
    hw = rows // nhalf        # columns per acc tile (512)
    fe = f + 1                # hp with ones column appended
    nchunk = max(1, rows // 512)  # chunks for hpT/srow row-form work

    act_jbs = default_act_jbs(kb, act_frac)

    # all inputs are pre-marshaled on host into device-native layouts so
    # every DMA is contiguous (>=512B descriptors, line rate)
    hb_d = nc.dram_tensor("hbb", [128, kb * f], F32, kind="ExternalInput")
    adjt_d = nc.dram_tensor("adjt", [keys, rows], U16, kind="ExternalInput")
    w_d = nc.dram_tensor("wt", [f, heads * f], F32, kind="ExternalInput")
    as_d = nc.dram_tensor("asT", [f, heads], F32, kind="ExternalInput")
    ad_d = nc.dram_tensor("adr", [1, heads * f], F32, kind="ExternalInput")
    out_d = nc.dram_tensor("out", [heads, rows, f], F32,
                           kind="ExternalOutput")

    pairs = [tuple(range(p, min(p + 2, heads)))
             for p in range(0, heads, 2)]

    with tile.TileContext(nc) as tc:
        with (
            tc.tile_pool(name="const", bufs=1) as const,
            tc.tile_pool(name="persist", bufs=1) as persist,
            tc.tile_pool(name="stmp", bufs=4) as stmp,
        ):
            id16 = const.tile([128, 128], F16, tag="id16")
            make_identity(nc, id16)
            id32 = const.tile([128, 128], F32, tag="id32")
            make_identity(nc, id32)

            # ---- global loads (all contiguous) --------------------------
            h32 = persist.tile([128, kb, f], F32, tag="h32")
            nc.sync.dma_start(
                out=h32, in_=hb_d.ap().rearrange("p (t f) -> p t f", f=f))
            h16 = persist.tile([128, kb, f], F16, tag="h16")
            nc.vector.tensor_copy(h16, h32)

            w32 = persist.tile([f, heads, f], F32, tag="w32")
            nc.sync.dma_start(
                out=w32, in_=w_d.ap().rearrange("f (h o) -> f h o", o=f))
            w16 = persist.tile([f, heads, f], F16, tag="w16")
            nc.vector.tensor_copy(w16, w32)

            # a_src transposed for row-form s matmuls: [f, heads]
            asT16 = persist.tile([f, heads], F16, tag="asT16")
            asT32 = persist.tile([f, heads], F32, tag="asT32")
            nc.sync.dma_start(out=asT32, in_=as_d.ap())
            nc.vector.tensor_copy(asT16, asT32)

            # a_dst broadcast for block-form d products: [128, heads, f]
            ad32 = persist.tile([1, heads, f], F32, tag="ad32")
            nc.sync.dma_start(
                out=ad32, in_=ad_d.ap().rearrange("p (h f) -> p h f", f=f))
            adb32 = persist.tile([128, heads, f], F32, tag="adb32")
            nc.gpsimd.partition_broadcast(adb32, ad32)
            adb16 = persist.tile([128, heads, f], F16, tag="adb16")
            nc.vector.tensor_copy(adb16, adb32)

            # ---- mask tiles: contiguous DMA of host-transposed adjT ------
            with tc.tile_pool(name="adjp", bufs=kb) as adjp:
                adjts = []
                for jb in range(kb):
                    adjt = adjp.tile([128, rows], U16, tag="adjt",
                                     name=f"adjt{jb}")
                    nc.sync.dma_start(
                        out=adjt,
                        in_=adjt_d.ap()[jb * 128:(jb + 1) * 128, :])
                    adjts.append(adjt)

                # ---- hT (transposed h, fp16) ----------------------------
                hT16 = persist.tile([64, keys], F16, tag="hT16")
                g_ht = min(4, kb)
                with tc.tile_pool(name="psum_ht", bufs=2,
                                  space="PSUM") as pht:
                    for g in range(kb // g_ht):
                        pt = pht.tile([64, g_ht * 128], F16, tag="pht")
                        for t in range(g_ht):
                            blk = g * g_ht + t
                            nc.tensor.transpose(
                                pt[:, t * 128:(t + 1) * 128],
                                h16[:, blk, :], id16)
                        nc.vector.tensor_copy(
                            hT16[:, g * g_ht * 128:(g + 1) * g_ht * 128], pt)

                # ---- per-head state --------------------------------------
                hpt = [None] * heads    # [128, kb, fe] fp16: [hp | 1]
                hpt2 = [None] * heads   # [128, kb, fe] fp16: v2-scaled
                vv = [None] * heads     # [128, kb] f32: e^d
                v2v = [None] * heads    # [128, kb] f32: e^{0.2 d}
                nv2c = [None] * heads   # [128, kb] f32: -c * e^{0.2 d}
                qbcp = [persist.tile([128, len(pr), rows], F16,
                                     tag=f"qbcp{pi}", name=f"qbcp{pi}")
                        for pi, pr in enumerate(pairs)]

                setup_pools = ExitStack()
                php = setup_pools.enter_context(
                    tc.tile_pool(name="psum_hp", bufs=2, space="PSUM"))
                rowp = setup_pools.enter_context(
                    tc.tile_pool(name="psum_row", bufs=1, space="PSUM"))

                def setup_head_steps(h, sd_eng, hpt2_eng):
                    """Emit-steps (closures) for head h's setup; callable one
                    at a time so pair-1 head setup can be spread across the
                    pair-0 main loop without clumping any engine's queue."""
                    pi, k = h // 2, h % 2
                    hpt_h = persist.tile([128, kb, fe], F16, tag=f"hpt{h}",
                                         name=f"hpt{h}")
                    tanh_h = stmp.tile([128, kb, f], F16, tag="tanh",
                                       name=f"tanh{h}")
                    v_h = persist.tile([128, kb], F32, tag=f"v{h}",
                                       name=f"v{h}")
                    v2_h = persist.tile([128, kb], F32, tag=f"v2{h}",
                                        name=f"v2{h}")
                    nv2_h = persist.tile([128, kb], F32, tag=f"nv2{h}",
                                         name=f"nv2{h}")
                    sums = stmp.tile([128, kb, 1], F32, tag="sums",
                                     name=f"sums{h}")
                    tanhT = stmp.tile([64, rows], F16, tag="tanhT",
                                      name=f"tanhT{h}")
                    qrow = stmp.tile([1, rows], F16, tag="qrow",
                                     name=f"qrow{h}")
                    hpt[h] = hpt_h
                    vv[h], v2v[h], nv2c[h] = v_h, v2_h, nv2_h

                    steps = []
                    g_hp = min(8, kb)

                    def hp_group(g):
                        pp = php.tile([128, g_hp * f], F32, tag="php",
                                      name=f"php{h}_{g}")
                        for t in range(g_hp):
                            blk = g * g_hp + t
                            nc.tensor.matmul(
                                pp[:, t * f:(t + 1) * f],
                                lhsT=hT16[:, blk * 128:(blk + 1) * 128],
                                rhs=w16[:, h, :], start=True, stop=True)
                        # hp to fp16 on DVE, tanh on ACT (splits the load)
                        nc.vector.tensor_copy(
                            hpt_h[:, g * g_hp:(g + 1) * g_hp, 0:f],
                            pp.rearrange("p (t o) -> p t o", o=f))
                        nc.scalar.activation(
                            tanh_h[:, g * g_hp:(g + 1) * g_hp, :],
                            pp.rearrange("p (t o) -> p t o", o=f),
                            ACTF.Tanh)

                    for g in range(kb // g_hp):
                        steps.append(lambda g=g: hp_group(g))

                    # s in row form: hpT = w^T @ h^T, tanh, a_src^T @ tanhT
                    # (early: it feeds the long qbc broadcast chain)
                    def s_chunk(cch):
                        sl = slice(cch * hw, (cch + 1) * hw)
                        hpTp = rowp.tile([64, hw], F32, tag="hpT",
                                         name=f"hpT{h}_{cch}")
                        nc.tensor.matmul(
                            hpTp, lhsT=w16[:, h, :], rhs=hT16[:, sl],
                            start=True, stop=True)
                        nc.scalar.activation(tanhT[:, sl], hpTp, ACTF.Tanh)
                        srowp = rowp.tile([1, hw], F32, tag="srow",
                                          name=f"srow{h}_{cch}")
                        nc.tensor.matmul(
                            srowp, lhsT=asT16[:, h:h + 1], rhs=tanhT[:, sl],
                            start=True, stop=True)
                        # q = e^{0.8 s}
                        nc.scalar.activation(qrow[:, sl], srowp, ACTF.Exp,
                                             scale=1.0 - NEG_SLOPE)

                    for cch in range(nchunk):
                        steps.append(lambda c=cch: s_chunk(c))
                    steps.append(lambda: nc.gpsimd.partition_broadcast(
                        qbcp[pi][:, k, :], qrow))

                    steps.append(
                        lambda: nc.vector.memset(hpt_h[:, :, f:fe], 1.0))

                    # d = tanh . a_dst per key (block/partition form)
                    def d_prod():
                        prod = stmp.tile([128, kb, f], F16, tag="prod",
                                         name=f"prod{h}")
                        sd_eng.tensor_tensor(
                            out=prod, in0=tanh_h,
                            in1=adb16[:, h].unsqueeze(1).broadcast_to(
                                [128, kb, f]),
                            op=ALU.mult)
                        nc.vector.reduce_sum(sums, prod, axis=AX.X)

                    steps.append(d_prod)

                    def d_exps():
                        nc.scalar.activation(v_h, sums[:, :, 0], ACTF.Exp)
                        nc.scalar.activation(v2_h, sums[:, :, 0], ACTF.Exp,
                                             scale=NEG_SLOPE)
                        nc.vector.tensor_scalar_mul(nv2_h, v2_h, -CMASK)

                    steps.append(d_exps)

                    # v2-scaled [hp | 1] for the act-path restore matmul
                    if act_jbs:
                        hpt2_h = persist.tile([128, kb, fe], F16,
                                              tag=f"hpt2{h}",
                                              name=f"hpt2{h}")
                        hpt2[h] = hpt2_h

                        def mk_hpt2():
                            hpt2_eng.tensor_tensor(
                                out=hpt2_h, in0=hpt_h,
                                in1=v2_h.unsqueeze(2).broadcast_to(
                                    [128, kb, fe]),
                                op=ALU.mult)

                        steps.append(mk_hpt2)
                    return steps

                for st in setup_head_steps(0, nc.vector, nc.vector):
                    st()
                for st in setup_head_steps(1, nc.vector, nc.vector):
                    st()
                late_steps = []

                # ---- main loop ------------------------------------------
                nacc = heads * nhalf
                acc_sb = persist.tile([fe, nacc, hw], F32, tag="acc_sb")

                with (
                    tc.tile_pool(name="mt", bufs=4) as mt,
                    tc.tile_pool(name="outp", bufs=4) as outp,
                ):
                    for pi, pair in enumerate(pairs):
                        np_ = len(pair)
                        acc_st = ExitStack()
                        accp = acc_st.enter_context(
                            tc.tile_pool(name=f"accp{pi}", bufs=1,
                                         space="PSUM"))
                        accs = {}
                        for h in pair:
                            for half in range(nhalf):
                                i = h * nhalf + half
                                accs[i] = accp.tile(
                                    [fe, hw], F32, tag=f"acc{i}",
                                    name=f"acc{i}")
                        for jb in range(kb):
                            if pi == 0 and jb == 1 and len(pairs) > 1:
                                # round-robin heads 2,3 so both chains
                                # advance together
                                s2 = setup_head_steps(2, nc.vector,
                                                      nc.gpsimd)
                                s3 = setup_head_steps(3, nc.vector,
                                                      nc.gpsimd)
                                for a, b in zip(s2, s3):
                                    late_steps += [a, b]
                                late_steps += s2[len(s3):] + s3[len(s2):]
                                nslots = max(1, min(6, kb - 2))
                                per_jb = -(-len(late_steps) // nslots)
                            if pi == 0 and jb >= 1:
                                # heads 2,3 setup rides under pair 0's loop,
                                # a few steps per iteration so no engine
                                # queue gets a long setup clump
                                for _ in range(per_jb):
                                    if late_steps:
                                        late_steps.pop(0)()
                            adj16 = adjts[jb].bitcast(F16)
                            is_act = jb in act_jbs
                            ta = mt.tile([128, np_, rows], F16, tag="ta")
                            tb = mt.tile([128, np_, rows], F16, tag="tb")
                            if is_act:
                                # X = q * adjT, then relu(v X - c v2) on ACT
                                nc.vector.tensor_tensor(
                                    out=ta, in0=qbcp[pi],
                                    in1=adj16.unsqueeze(1).broadcast_to(
                                        [128, np_, rows]),
                                    op=ALU.mult)
                                for k, h in enumerate(pair):
                                    nc.scalar.activation(
                                        tb[:, k, :], ta[:, k, :], ACTF.Relu,
                                        bias=nv2c[h][:, jb:jb + 1],
                                        scale=vv[h][:, jb:jb + 1])
                            else:
                                for k, h in enumerate(pair):
                                    nc.vector.tensor_scalar(
                                        out=ta[:, k, :], in0=qbcp[pi][:, k, :],
                                        scalar1=vv[h][:, jb:jb + 1],
                                        scalar2=v2v[h][:, jb:jb + 1],
                                        op0=ALU.mult, op1=ALU.max)
                                nc.vector.tensor_tensor(
                                    out=tb, in0=ta,
                                    in1=adj16.unsqueeze(1).broadcast_to(
                                        [128, np_, rows]),
                                    op=ALU.mult)
                            for k, h in enumerate(pair):
                                for half in range(nhalf):
                                    sl = slice(half * hw, (half + 1) * hw)
                                    nc.tensor.matmul(
                                        accs[h * nhalf + half],
                                        lhsT=hpt[h][:, jb, :],
                                        rhs=tb[:, k, sl],
                                        start=(jb == 0),
                                        stop=(jb == kb - 1 and not is_act))
                                    if is_act:
                                        nc.tensor.matmul(
                                            accs[h * nhalf + half],
                                            lhsT=hpt2[h][:, jb, :],
                                            rhs=adj16[:, sl],
                                            start=False,
                                            stop=(jb == kb - 1))

                        while late_steps:
                            late_steps.pop(0)()
                        # spill this pair's accumulators to SBUF on ACT
                        for h in pair:
                            for half in range(nhalf):
                                i = h * nhalf + half
                                nc.scalar.activation(
                                    acc_sb[:, i, :], accs[i], ACTF.Identity)
                        acc_st.close()
                        if pi == 0:
                            # setup psum pools (under accp0 on the stack) can
                            # only pop after accp0 does
                            setup_pools.close()
                        # normalize in transposed [i, o] form
                        nq = hw // 128
                        ptf_st = ExitStack()
                        ptf = ptf_st.enter_context(
                            tc.tile_pool(name=f"ptf{pi}", bufs=2,
                                         space="PSUM"))
                        for h in pair:
                            for half in range(nhalf):
                                i = h * nhalf + half
                                pt = ptf.tile([128, nq, fe], F32,
                                              tag=f"pt{pi}")
                                for q in range(nq):
                                    nc.tensor.transpose(
                                        pt[:, q, :],
                                        acc_sb[:, i, q * 128:(q + 1) * 128],
                                        id32[0:fe, 0:fe])
                                rcol = outp.tile([128, nq], F32, tag="rcol")
                                nc.vector.reciprocal(rcol, pt[:, :, f])
                                osb = outp.tile([128, nq, f], F32, tag="osb")
                                nc.vector.tensor_tensor(
                                    out=osb, in0=pt[:, :, 0:f],
                                    in1=rcol.unsqueeze(2).broadcast_to(
                                        [128, nq, f]),
                                    op=ALU.mult)
                                nc.sync.dma_start(
                                    out=out_d.ap()[
                                        h, half * hw:(half + 1) * hw, :]
                                    .rearrange("(q p) f -> p q f", p=128),
                                    in_=osb)
                        ptf_st.close()
    nc.compile()
    return nc


_PROGRAM_CACHE = {}


def _get_program():
    key = "full"
    if key not in _PROGRAM_CACHE:
        _PROGRAM_CACHE[key] = build_program()
    return _PROGRAM_CACHE[key]


def make_in_maps(h, adj, w, a_src, a_dst):
    """Shard + marshal the full inputs into 8 per-core input maps."""
    h = np.ascontiguousarray(np.asarray(h, dtype=np.float32))
    adj = np.ascontiguousarray(np.asarray(adj, dtype=np.float32))
    w = np.ascontiguousarray(np.asarray(w, dtype=np.float32))
    a_s = np.asarray(a_src, np.float32)[:, :, 0]   # [H, F]
    a_d = np.asarray(a_dst, np.float32)[:, :, 0]   # [H, F]
    wt = np.ascontiguousarray(
        w.transpose(1, 0, 2).reshape(F, H * F))     # [F, H*F]
    asT = np.ascontiguousarray(a_s.T)               # [F, H]
    adr = np.ascontiguousarray(a_d.reshape(1, H * F))
    kb = KEYS // 128
    in_maps = []
    for c in range(NCORES):
        b, r0 = c // 2, (c % 2) * ROWS
        hb = np.concatenate([h[b, r0:], h[b, :r0]], axis=0)  # rotate keys
        hbb = np.ascontiguousarray(
            hb.reshape(kb, 128, F).transpose(1, 0, 2).reshape(128, kb * F))
        adj_rows = adj[b, r0:r0 + ROWS]
        adj_rot = np.concatenate([adj_rows[:, r0:], adj_rows[:, :r0]], axis=1)
        adjt_f = np.ascontiguousarray(adj_rot.T)  # [KEYS, ROWS] f32
        adjt = np.ascontiguousarray(
            adjt_f.view(np.uint16).reshape(KEYS, ROWS, 2)[:, :, 1])
        in_maps.append({
            "hbb": hbb,
            "adjt": adjt,
            "wt": wt,
            "asT": asT,
            "adr": adr,
        })
    return in_maps


def assemble_output(results, bias):
    """Gather per-core [H, ROWS, F] results into [B, H, N, F]."""
    out = np.empty((B, H, N, F), dtype=np.float32)
    for c in range(NCORES):
        b, r0 = c // 2, (c % 2) * ROWS
        out[b, :, r0:r0 + ROWS, :] = results[c]["out"]
    if bias is not None:
        out = out + np.asarray(bias, dtype=np.float32)[None, None, None, :]
    return out


def run(h, adj, w, a_src, a_dst, bias, trace=False, trace_kwargs=None):
    nc = _get_program()
    in_maps = make_in_maps(h, adj, w, a_src, a_dst)
    res = run_bass_kernel_spmd(nc, in_maps, core_ids=list(range(NCORES)),
                               trace=trace, **(trace_kwargs or {}))
    return assemble_output(res.results, bias), res


def kernel(h, adj, w, a_src, a_dst, bias):
    out, _ = run(h, adj, w, a_src, a_dst, bias,
                 trace=bool(int(os.environ.get("GAT_TRACE", "0"))))
    return out
